# revision 1
# baseline (speedup 1.0000x reference)
"""KSparseFFTClassifier Trainium2 kernel.

Math: reference computes
    h   = x @ W_proj.T + b_proj                      (bs, 129)
    h  *= scale  (sqrt(2) on dims 1..64)
    out = IDFT65(h[:, :65]) + h[:, 65:] @ Ws.T       (bs, 16384)

The zero-padded orthonormal IDFT of the 65 nonzero frequency components is a
dense matmul against a (65, N) cos/sin basis; the DC row of that basis is the
constant 1/sqrt(N).  So with M = [scaled cos/sin basis for h dims 1..64;
Ws.T]  (128 x N):

    out[b, n] = h[b, 1:129] @ M[:, n] + (h[b, 0] + 0) / sqrt(N)

i.e. a (bs,2048)x(2048,128) matmul, a (bs,128)x(128,N) matmul, and a
per-row scalar (the DC term) added during PSUM eviction.

Sharding: data-parallel over batch, 512 rows per core on 8 cores.
"""

import numpy as np

BS = 4096
IN_DIM = 2048
N = 16384
K = 32
SLACK = 64
NCORES = 8
BC = BS // NCORES        # 512 batch rows per core
P = 128
KT = IN_DIM // P         # 16 contraction tiles for matmul1
NCHUNK = 4096            # output column chunk (SBUF out tile free size)
NCH = N // NCHUNK        # 4

# matmul dtypes ("float32" = exact 2-pass fp32, 4 cyc/row;
# "float32r" = single-pass fp32, 1 cyc/row at free>=256)
MM1_DT = "float32r"
MM2_DT = "float32r"

_NC_CACHE = {}


def _build_nc(mm1_name, mm2_name):
    import concourse.bacc as bacc
    import concourse.mybir as mybir
    import concourse.tile as tile

    f32 = mybir.dt.float32
    mm1 = getattr(mybir.dt, mm1_name)
    mm2 = getattr(mybir.dt, mm2_name)

    nc = bacc.Bacc("TRN2", target_bir_lowering=False)

    xT = nc.dram_tensor("xT", [P, KT * BC], mm1, kind="ExternalInput")
    w1t = nc.dram_tensor("w1t", [P, KT * P], mm1, kind="ExternalInput")
    w0 = nc.dram_tensor("w0", [P, KT], f32, kind="ExternalInput")
    mmat = nc.dram_tensor("mmat", [P, N], mm2, kind="ExternalInput")
    bt = nc.dram_tensor("bt", [P, 1], f32, kind="ExternalInput")
    cst = nc.dram_tensor("cst", [1, 1], f32, kind="ExternalInput")
    out = nc.dram_tensor("out", [BC, N], f32, kind="ExternalOutput")

    Ident = mybir.ActivationFunctionType.Identity

    with tile.TileContext(nc) as tc:
        with (
            tc.tile_pool(name="wp", bufs=1) as wp,
            tc.tile_pool(name="xp", bufs=1) as xp,
            tc.tile_pool(name="mp", bufs=1) as mp,
            tc.tile_pool(name="hp", bufs=1) as hp,
            tc.tile_pool(name="op", bufs=3) as op,
            tc.tile_pool(name="ps", bufs=4, space="PSUM") as ps,
            tc.tile_pool(name="ps1", bufs=1, space="PSUM") as ps1,
            tc.tile_pool(name="ps2", bufs=1, space="PSUM") as ps2,
        ):
            w1t_sb = wp.tile([P, KT * P], mm1, tag="w1t")
            nc.sync.dma_start(out=w1t_sb[:, :], in_=w1t[:, :])
            w0_sb = wp.tile([P, KT], f32, tag="w0")
            nc.sync.dma_start(out=w0_sb[:, :], in_=w0[:, :])
            bt_sb = wp.tile([P, 1], f32, tag="bt")
            nc.sync.dma_start(out=bt_sb[:, :], in_=bt[:, :])
            cst_sb = wp.tile([1, 1], f32, tag="cst")
            nc.sync.dma_start(out=cst_sb[:, :], in_=cst[:, :])
            ones_sb = wp.tile([1, 1], f32, tag="ones")
            nc.vector.memset(ones_sb[:, :], 1.0)

            # x transposed, packed on host as 4 groups of 4 k-tiles
            xg = []
            for g in range(4):
                t = xp.tile([P, 4 * BC], mm1, tag=f"xg{g}")
                nc.sync.dma_start(out=t[:, :], in_=xT[:, g * 4 * BC:(g + 1) * 4 * BC])
                xg.append(t)

            # combined IDFT-basis + Ws.T matrix, resident in SBUF
            mm = []
            for ti in range(NCH):
                m = mp.tile([P, NCHUNK], mm2, tag=f"m{ti}")
                nc.sync.dma_start(out=m[:, :], in_=mmat[:, ti * NCHUNK:(ti + 1) * NCHUNK])
                mm.append(m)

            # matmul1: hT[d, b] for d = h dims 1..128
            hT_ps = ps1.tile([P, BC], f32, tag="hT")
            for kt in range(KT):
                nc.tensor.matmul(
                    hT_ps[:, :],
                    lhsT=w1t_sb[:, kt * P:(kt + 1) * P],
                    rhs=xg[kt // 4][:, (kt % 4) * BC:(kt % 4 + 1) * BC],
                    start=(kt == 0),
                    stop=(kt == KT - 1),
                )
            hT_sb = hp.tile([P, BC], mm2, tag="hT_sb")
            nc.scalar.add(hT_sb[:, :], hT_ps[:, :], bt_sb[:, 0:1])

            # dc row: h dim 0 (as (1, BC)), then PE-transpose to (P, 4)
            dcr_ps = ps2.tile([1, BC], f32, tag="dcr")
            for kt in range(KT):
                nc.tensor.matmul(
                    dcr_ps[:, :],
                    lhsT=w0_sb[:, kt:kt + 1],
                    rhs=xg[kt // 4][:, (kt % 4) * BC:(kt % 4 + 1) * BC].bitcast(f32),
                    start=(kt == 0),
                    stop=(kt == KT - 1),
                )
            dcr_sb = hp.tile([1, BC], f32, tag="dcr_sb")
            nc.scalar.activation(
                dcr_sb[:, :], dcr_ps[:, :], Ident,
                bias=cst_sb[0:1, 0:1], scale=float(1.0 / np.sqrt(N)),
            )
            dc_sb = hp.tile([P, BC // P], f32, tag="dc_sb")
            for j in range(BC // P):
                dcc_ps = ps2.tile([P, 1], f32, tag="dcc")
                nc.tensor.matmul(
                    dcc_ps[:, :],
                    lhsT=dcr_sb[0:1, j * P:(j + 1) * P],
                    rhs=ones_sb[0:1, 0:1],
                    start=True,
                    stop=True,
                )
                nc.scalar.copy(dc_sb[:, j:j + 1], dcc_ps[:, :])

            # matmul2 + DC bias-add eviction + store
            ev = 0
            for ti in range(NCH):
                for j in range(BC // P):
                    ob = op.tile([P, NCHUNK], f32, tag="ob")
                    for s in range(NCHUNK // 512):
                        pt = ps.tile([P, 512], f32, tag="mm2")
                        nc.tensor.matmul(
                            pt[:, :],
                            lhsT=hT_sb[:, j * P:(j + 1) * P],
                            rhs=mm[ti][:, s * 512:(s + 1) * 512],
                            start=True,
                            stop=True,
                        )
                        dst = ob[:, s * 512:(s + 1) * 512]
                        if ev % 2 == 0:
                            nc.scalar.add(dst, pt[:, :], dc_sb[:, j:j + 1])
                        else:
                            nc.vector.tensor_scalar_add(dst, pt[:, :], dc_sb[:, j:j + 1])
                        ev += 1
                    nc.sync.dma_start(
                        out=out[j * P:(j + 1) * P, ti * NCHUNK:(ti + 1) * NCHUNK],
                        in_=ob[:, :],
                    )
    nc.compile()
    return nc


def _get_nc():
    key = (MM1_DT, MM2_DT)
    if key not in _NC_CACHE:
        _NC_CACHE[key] = _build_nc(*key)
    return _NC_CACHE[key]


def _host_pack(x, W_proj, b_proj, Ws):
    SQRT2 = np.float64(np.sqrt(np.float32(2.0)))
    n_idx = np.arange(N, dtype=np.float64)
    k_idx = np.arange(1, K + 1, dtype=np.float64)
    theta = (2.0 * np.pi / N) * np.outer(k_idx, n_idx)
    M = np.empty((P, N), np.float32)
    isqn = 1.0 / np.sqrt(np.float64(N))
    M[0:2 * K:2] = (SQRT2 * isqn) * np.cos(theta)
    M[1:2 * K:2] = (SQRT2 * isqn) * np.sin(theta)
    M[2 * K:] = Ws.T

    w1 = W_proj[1:P + 1]                                  # (128, 2048)
    w1t = np.ascontiguousarray(
        w1.T.reshape(KT, P, P).transpose(1, 0, 2).reshape(P, KT * P)
    )
    w0 = np.ascontiguousarray(W_proj[0].reshape(KT, P).T)  # (128, 16)
    bt = np.ascontiguousarray(b_proj[1:P + 1].reshape(P, 1))
    cst = np.asarray(b_proj[0] / np.sqrt(np.float64(N)), np.float32).reshape(1, 1)

    xts = []
    for c in range(NCORES):
        xc = x[c * BC:(c + 1) * BC]                        # (512, 2048)
        xt = np.ascontiguousarray(
            xc.T.reshape(KT, P, BC).transpose(1, 0, 2).reshape(P, KT * BC)
        )
        xts.append(xt)
    return M, w1t, w0, bt, cst, xts


def kernel(x, W_proj, b_proj, Ws, _trace=False, _tmpdir=None):
    from concourse import bass_utils

    x = np.ascontiguousarray(x, np.float32)
    W_proj = np.ascontiguousarray(W_proj, np.float32)
    b_proj = np.ascontiguousarray(b_proj, np.float32)
    Ws = np.ascontiguousarray(Ws, np.float32)

    M, w1t, w0, bt, cst, xts = _host_pack(x, W_proj, b_proj, Ws)
    nc = _get_nc()

    in_maps = [
        {"xT": xts[c], "w1t": w1t, "w0": w0, "mmat": M, "bt": bt, "cst": cst}
        for c in range(NCORES)
    ]
    kw = {}
    if _trace:
        kw = dict(trace=True, tmpdir=_tmpdir, trace_cores=[0])
    res = bass_utils.run_bass_kernel_spmd(nc, in_maps, core_ids=list(range(NCORES)), **kw)
    out = np.concatenate([r["out"] for r in res.results], axis=0)
    if _trace:
        return out, res
    return out



# revision 2
# speedup vs baseline: 1.6521x; 1.6521x over previous
"""KSparseFFTClassifier Trainium2 kernel.

Math: reference computes
    h   = x @ W_proj.T + b_proj                      (bs, 129)
    h  *= scale  (sqrt(2) on dims 1..64)
    out = IDFT65(h[:, :65]) + h[:, 65:] @ Ws.T       (bs, 16384)

The zero-padded orthonormal IDFT of the 65 nonzero frequency components is a
dense matmul against a (65, N) cos/sin basis; the DC row of that basis is the
constant 1/sqrt(N).  So with M = [scaled cos/sin basis for h dims 1..64;
Ws.T]  (128 x N):

    out[b, n] = h[b, 1:129] @ M[:, n] + (h[b, 0] + 0) / sqrt(N)

i.e. a (bs,2048)x(2048,128) matmul, a (bs,128)x(128,N) matmul, and a
per-row scalar (the DC term) added during PSUM eviction.

Sharding: data-parallel over batch, 512 rows per core on 8 cores.

The kernel is DMA-bound (output is 512x16384 f32 = 33.5 MB/core at 360 GB/s);
all tensors move over the bus as fp16 (error budget 2e-2, fp16 gives ~7e-4)
and the host up-casts the fp16 output shard back to f32.
"""

import numpy as np

BS = 4096
IN_DIM = 2048
N = 16384
K = 32
SLACK = 64
NCORES = 8
BC = BS // NCORES        # 512 batch rows per core
P = 128
KT = IN_DIM // P         # 16 contraction tiles for matmul1
NCHUNK = 4096            # output column chunk (SBUF out tile free size)
NCH = N // NCHUNK        # 4

MM1_DT = "float16"
MM2_DT = "float16"

_NC_CACHE = {}


def _build_nc(mm1_name, mm2_name):
    import concourse.bacc as bacc
    import concourse.mybir as mybir
    import concourse.tile as tile

    f32 = mybir.dt.float32
    mm1 = getattr(mybir.dt, mm1_name)
    mm2 = getattr(mybir.dt, mm2_name)

    nc = bacc.Bacc("TRN2", target_bir_lowering=False)

    xT = nc.dram_tensor("xT", [P, KT * BC], mm1, kind="ExternalInput")
    w1t = nc.dram_tensor("w1t", [P, KT * P], mm1, kind="ExternalInput")
    w0 = nc.dram_tensor("w0", [P, KT], mm1, kind="ExternalInput")
    mmat = nc.dram_tensor("mmat", [P, N], mm2, kind="ExternalInput")
    bt = nc.dram_tensor("bt", [P, 1], f32, kind="ExternalInput")
    cst = nc.dram_tensor("cst", [1, 1], f32, kind="ExternalInput")
    out = nc.dram_tensor("out", [BC, N], mm2, kind="ExternalOutput")

    Ident = mybir.ActivationFunctionType.Identity

    with tile.TileContext(nc) as tc:
        with (
            tc.tile_pool(name="wp", bufs=1) as wp,
            tc.tile_pool(name="xp", bufs=1) as xp,
            tc.tile_pool(name="mp", bufs=1) as mp,
            tc.tile_pool(name="hp", bufs=1) as hp,
            tc.tile_pool(name="op", bufs=3) as op,
            tc.tile_pool(name="ps", bufs=4, space="PSUM") as ps,
            tc.tile_pool(name="ps1", bufs=1, space="PSUM") as ps1,
            tc.tile_pool(name="ps2", bufs=1, space="PSUM") as ps2,
        ):
            w1t_sb = wp.tile([P, KT * P], mm1, tag="w1t")
            nc.sync.dma_start(out=w1t_sb[:, :], in_=w1t[:, :])
            w0_sb = wp.tile([P, KT], mm1, tag="w0")
            nc.sync.dma_start(out=w0_sb[:, :], in_=w0[:, :])
            bt_sb = wp.tile([P, 1], f32, tag="bt")
            nc.sync.dma_start(out=bt_sb[:, :], in_=bt[:, :])
            cst_sb = wp.tile([1, 1], f32, tag="cst")
            nc.sync.dma_start(out=cst_sb[:, :], in_=cst[:, :])
            ones_sb = wp.tile([1, 1], f32, tag="ones")
            nc.vector.memset(ones_sb[:, :], 1.0)

            # x transposed, packed on host as 4 groups of 4 k-tiles
            xg = []
            for g in range(4):
                t = xp.tile([P, 4 * BC], mm1, tag=f"xg{g}")
                nc.sync.dma_start(out=t[:, :], in_=xT[:, g * 4 * BC:(g + 1) * 4 * BC])
                xg.append(t)

            # combined IDFT-basis + Ws.T matrix, resident in SBUF
            mm = []
            for ti in range(NCH):
                m = mp.tile([P, NCHUNK], mm2, tag=f"m{ti}")
                nc.sync.dma_start(out=m[:, :], in_=mmat[:, ti * NCHUNK:(ti + 1) * NCHUNK])
                mm.append(m)

            # matmul1: hT[d, b] for d = h dims 1..128
            hT_ps = ps1.tile([P, BC], f32, tag="hT")
            for kt in range(KT):
                nc.tensor.matmul(
                    hT_ps[:, :],
                    lhsT=w1t_sb[:, kt * P:(kt + 1) * P],
                    rhs=xg[kt // 4][:, (kt % 4) * BC:(kt % 4 + 1) * BC],
                    start=(kt == 0),
                    stop=(kt == KT - 1),
                )
            hT_sb = hp.tile([P, BC], mm2, tag="hT_sb")
            nc.scalar.add(hT_sb[:, :], hT_ps[:, :], bt_sb[:, 0:1])

            # dc row: h dim 0 (as (1, BC)), then PE-transpose to (P, 4)
            dcr_ps = ps2.tile([1, BC], f32, tag="dcr")
            for kt in range(KT):
                nc.tensor.matmul(
                    dcr_ps[:, :],
                    lhsT=w0_sb[:, kt:kt + 1],
                    rhs=xg[kt // 4][:, (kt % 4) * BC:(kt % 4 + 1) * BC],
                    start=(kt == 0),
                    stop=(kt == KT - 1),
                )
            dcr_sb = hp.tile([1, BC], f32, tag="dcr_sb")
            nc.scalar.activation(
                dcr_sb[:, :], dcr_ps[:, :], Ident,
                bias=cst_sb[0:1, 0:1], scale=float(1.0 / np.sqrt(N)),
            )
            dc_sb = hp.tile([P, BC // P], f32, tag="dc_sb")
            for j in range(BC // P):
                dcc_ps = ps2.tile([P, 1], f32, tag="dcc")
                nc.tensor.matmul(
                    dcc_ps[:, :],
                    lhsT=dcr_sb[0:1, j * P:(j + 1) * P],
                    rhs=ones_sb[0:1, 0:1],
                    start=True,
                    stop=True,
                )
                nc.scalar.copy(dc_sb[:, j:j + 1], dcc_ps[:, :])

            # matmul2 + DC bias-add eviction + store
            ev = 0
            for ti in range(NCH):
                for j in range(BC // P):
                    ob = op.tile([P, NCHUNK], mm2, tag="ob")
                    for s in range(NCHUNK // 512):
                        pt = ps.tile([P, 512], f32, tag="mm2")
                        nc.tensor.matmul(
                            pt[:, :],
                            lhsT=hT_sb[:, j * P:(j + 1) * P],
                            rhs=mm[ti][:, s * 512:(s + 1) * 512],
                            start=True,
                            stop=True,
                        )
                        dst = ob[:, s * 512:(s + 1) * 512]
                        if ev % 2 == 0:
                            nc.scalar.add(dst, pt[:, :], dc_sb[:, j:j + 1])
                        else:
                            nc.vector.tensor_scalar_add(dst, pt[:, :], dc_sb[:, j:j + 1])
                        ev += 1
                    nc.sync.dma_start(
                        out=out[j * P:(j + 1) * P, ti * NCHUNK:(ti + 1) * NCHUNK],
                        in_=ob[:, :],
                    )
    nc.compile()
    return nc


def _get_nc():
    key = (MM1_DT, MM2_DT)
    if key not in _NC_CACHE:
        _NC_CACHE[key] = _build_nc(*key)
    return _NC_CACHE[key]


def _np_dt(name):
    import ml_dtypes
    return {"float16": np.float16, "bfloat16": ml_dtypes.bfloat16,
            "float32": np.float32, "float32r": np.float32}[name]


def _host_pack(x, W_proj, b_proj, Ws):
    dt1 = _np_dt(MM1_DT)
    dt2 = _np_dt(MM2_DT)
    SQRT2 = np.float64(np.sqrt(np.float32(2.0)))
    n_idx = np.arange(N, dtype=np.float64)
    k_idx = np.arange(1, K + 1, dtype=np.float64)
    theta = (2.0 * np.pi / N) * np.outer(k_idx, n_idx)
    M = np.empty((P, N), np.float32)
    isqn = 1.0 / np.sqrt(np.float64(N))
    M[0:2 * K:2] = (SQRT2 * isqn) * np.cos(theta)
    M[1:2 * K:2] = (SQRT2 * isqn) * np.sin(theta)
    M[2 * K:] = Ws.T
    M = M.astype(dt2)

    w1 = W_proj[1:P + 1]                                  # (128, 2048)
    w1t = np.ascontiguousarray(
        w1.T.reshape(KT, P, P).transpose(1, 0, 2).reshape(P, KT * P)
    ).astype(dt1)
    w0 = np.ascontiguousarray(W_proj[0].reshape(KT, P).T).astype(dt1)  # (128, 16)
    bt = np.ascontiguousarray(b_proj[1:P + 1].reshape(P, 1))
    cst = np.asarray(b_proj[0] / np.sqrt(np.float64(N)), np.float32).reshape(1, 1)

    xts = []
    for c in range(NCORES):
        xc = x[c * BC:(c + 1) * BC]                        # (512, 2048)
        xt = np.ascontiguousarray(
            xc.T.reshape(KT, P, BC).transpose(1, 0, 2).reshape(P, KT * BC)
        ).astype(dt1)
        xts.append(xt)
    return M, w1t, w0, bt, cst, xts


def kernel(x, W_proj, b_proj, Ws, _trace=False, _tmpdir=None):
    from concourse import bass_utils

    x = np.ascontiguousarray(x, np.float32)
    W_proj = np.ascontiguousarray(W_proj, np.float32)
    b_proj = np.ascontiguousarray(b_proj, np.float32)
    Ws = np.ascontiguousarray(Ws, np.float32)

    M, w1t, w0, bt, cst, xts = _host_pack(x, W_proj, b_proj, Ws)
    nc = _get_nc()

    in_maps = [
        {"xT": xts[c], "w1t": w1t, "w0": w0, "mmat": M, "bt": bt, "cst": cst}
        for c in range(NCORES)
    ]
    kw = {}
    if _trace:
        kw = dict(trace=True, tmpdir=_tmpdir, trace_cores=[0])
    res = bass_utils.run_bass_kernel_spmd(nc, in_maps, core_ids=list(range(NCORES)), **kw)
    out = np.concatenate([r["out"] for r in res.results], axis=0).astype(np.float32)
    if _trace:
        return out, res
    return out
